# revision 1
# baseline (speedup 1.0000x reference)
"""BAM-style attention block (avgpool8 -> 1024-token attention -> nearest-upsample + residual)
as a distributed Bass kernel on 8 TRN2 NeuronCores.

Sharding: core = b*2 + half  (b = batch 0..3, half = H-half 0..1).
Each core:
  phase 1: streams its x shard [512, 128, 256] per 128-channel group (sync ring),
           avg-pools 8x8 on DVE, and pipelines a pairwise AllGather of each pooled
           group (gpsimd ring) with the streaming
  phase 2: q/k/v projections + 512x1024 attention (bf16) in local-first token
           order; the local-token half of the attention runs while the last
           collective is still in flight, softmax normalization is deferred to a
           final row-sum rescale of y
  phase 3: re-streams x (sync ring), adds the upsampled attention output on DVE,
           writes out (scalar ring)
"""

import os
import numpy as np

B, C, H, W = 4, 512, 256, 256
DS = 8
HL = H // 2            # 128 rows per core
IL = HL // DS          # 16 pooled rows per core
WP = W // DS           # 32 pooled cols
NLOC = IL * WP         # 512 local tokens
N = 2 * NLOC           # 1024 tokens
K = C // 8             # 64
CG = C // 128          # 4 channel groups
NCHUNK = 8             # phase-3 chunks per channel group (2 pooled rows each)
ROWS_PER_CHUNK = 16    # = 2 * DS
NT = N // 128          # 8 token tiles (0..3 local, 4..7 remote)

_CACHE = {}
TRACE = bool(int(os.environ.get("BAM_TRACE", "0")))
LAST_EXEC_NS = None


def _build():
    import concourse.bass as bass
    import concourse.tile as tile
    from concourse import bacc, mybir
    from concourse.masks import make_identity

    f32 = mybir.dt.float32
    bf16 = mybir.dt.bfloat16
    ADD = mybir.AluOpType.add
    SUB = mybir.AluOpType.subtract
    MUL = mybir.AluOpType.mult
    AXY = mybir.AxisListType.XY
    Exp = mybir.ActivationFunctionType.Exp
    POOL_SCALE = 1.0 / (DS * DS)

    nc = bacc.Bacc("TRN2", target_bir_lowering=False, debug=False, num_devices=8)

    x_ext = nc.dram_tensor("x", [C, HL, W], f32, kind="ExternalInput")
    wq_ext = nc.dram_tensor("wq", [K, C], f32, kind="ExternalInput")
    bq_ext = nc.dram_tensor("bq", [1, K], f32, kind="ExternalInput")
    wk_ext = nc.dram_tensor("wk", [K, C], f32, kind="ExternalInput")
    bk_ext = nc.dram_tensor("bk", [1, K], f32, kind="ExternalInput")
    wv_ext = nc.dram_tensor("wv", [C, C], f32, kind="ExternalInput")
    bv_ext = nc.dram_tensor("bv", [1, C], f32, kind="ExternalInput")
    out_ext = nc.dram_tensor("out", [C, HL, W], f32, kind="ExternalOutput")

    with tile.TileContext(nc) as tc:
        with tc.tile_pool(name="persist", bufs=1) as persist, \
             tc.tile_pool(name="scratch", bufs=2) as scratch, \
             tc.tile_pool(name="p1", bufs=4) as p1, \
             tc.tile_pool(name="p3", bufs=5) as p3, \
             tc.tile_pool(name="psA", bufs=4, space="PSUM") as psA, \
             tc.tile_pool(name="psY", bufs=1, space="PSUM") as psY, \
             tc.tile_pool(name="dram", bufs=1, space="DRAM") as dram:

            # ---- constants & weights (scalar-engine DMA ring; PE transposes) ----
            ident = persist.tile([128, 128], bf16, tag="ident")
            make_identity(nc, ident[:])
            ones = persist.tile([1, N], bf16, tag="ones")
            nc.vector.memset(ones[:], 1.0)
            ones_col = persist.tile([128, 1], bf16, tag="ones_col")
            nc.vector.memset(ones_col[:], 1.0)
            ones_f32 = persist.tile([1, 128], f32, tag="ones_f32")
            nc.vector.memset(ones_f32[:], 1.0)

            def load_bias(ext, n):
                st = scratch.tile([1, n], f32, tag="bstage")
                nc.scalar.dma_start(out=st[:], in_=ext.ap())
                bb = persist.tile([1, n], bf16, tag=f"b_{ext.name}", name=f"b_{ext.name}")
                nc.scalar.copy(out=bb[:], in_=st[:])
                return bb

            bq_b = load_bias(bq_ext, K)
            bk_b = load_bias(bk_ext, K)
            bv_b = load_bias(bv_ext, C)

            def load_qk_weight(ext):
                st = scratch.tile([K, C], f32, tag="wstage")
                nc.scalar.dma_start(out=st[:], in_=ext.ap())
                wb = persist.tile([K, C], bf16, tag=f"wb_{ext.name}", name=f"wb_{ext.name}")
                nc.scalar.copy(out=wb[:], in_=st[:])
                wT = []
                for cg in range(CG):
                    ps = psA.tile([128, K], bf16, tag="s")
                    nc.tensor.transpose(ps[:], wb[:, cg * 128:(cg + 1) * 128],
                                        ident[0:K, 0:K])
                    t = persist.tile([128, K], bf16, tag=f"wT_{ext.name}{cg}",
                                     name=f"wT_{ext.name}{cg}")
                    nc.scalar.copy(out=t[:], in_=ps[:])
                    wT.append(t)
                return wT

            wqT = load_qk_weight(wq_ext)
            wkT = load_qk_weight(wk_ext)

            # wvT[cg][c_loc, d] = Wv[d, cg*128 + c_loc]
            wvT = [persist.tile([128, C], bf16, tag=f"wvT{cg}", name=f"wvT{cg}")
                   for cg in range(CG)]
            for dt in range(CG):
                st = scratch.tile([128, C], f32, tag="wstage")
                nc.scalar.dma_start(out=st[:], in_=wv_ext.ap()[dt * 128:(dt + 1) * 128, :])
                wvb = scratch.tile([128, C], bf16, tag="wvstage")
                nc.scalar.copy(out=wvb[:], in_=st[:])
                for cg in range(CG):
                    ps = psA.tile([128, 128], bf16, tag="s")
                    nc.tensor.transpose(ps[:], wvb[:, cg * 128:(cg + 1) * 128], ident[:])
                    nc.scalar.copy(out=wvT[cg][:, dt * 128:(dt + 1) * 128], in_=ps[:])

            # ---- phase 1: stream x + avg-pool; per-cg exchange on the gpsimd ring ----
            # Tokens are kept LOCAL-FIRST through phase 2: columns [0:512] are this
            # core's tokens, [512:1024] the partner's. Softmax and the final
            # contraction are permutation-invariant over n, so the global order is
            # never materialized.
            xf = [persist.tile([128, NLOC], f32, tag=f"xf{cg}", name=f"xf{cg}")
                  for cg in range(CG)]
            xfb_loc = [persist.tile([128, NLOC], bf16, tag=f"xfl{cg}", name=f"xfl{cg}")
                       for cg in range(CG)]
            xfb_rem = [persist.tile([128, NLOC], bf16, tag=f"xfr{cg}", name=f"xfr{cg}")
                       for cg in range(CG)]
            xf_loc_d = dram.tile([CG, 128, NLOC], f32, tag="xf_loc")
            xf_all_d = dram.tile([CG, 2, 128, NLOC], f32, tag="xf_all")

            q_ps = psA.tile([K, NLOC], f32, tag="s")
            kl_ps = psA.tile([K, NLOC], f32, tag="s")
            kr_ps = psA.tile([K, NLOC], f32, tag="s")

            def remote_recover(cg):
                # partner half = (h0 + h1) - local, recovered rank-agnostically
                xfg = scratch.tile([128, N], f32, tag="xfg", name=f"xfg{cg}")
                for hf in range(2):
                    nc.gpsimd.dma_start(out=xfg[:, hf * NLOC:(hf + 1) * NLOC],
                                        in_=xf_all_d[cg, hf])
                hsum = scratch.tile([128, NLOC], f32, tag="hsum", bufs=1,
                                    name=f"hsum{cg}")
                nc.gpsimd.tensor_tensor(out=hsum[:], in0=xfg[:, :NLOC],
                                        in1=xfg[:, NLOC:], op=ADD)
                rem_raw = scratch.tile([128, NLOC], f32, tag="rem_raw", bufs=1,
                                       name=f"rem_raw{cg}")
                nc.gpsimd.tensor_tensor(out=rem_raw[:], in0=hsum[:],
                                        in1=xf[cg][:], op=SUB)
                # on ACT: a collective-latency stall here must not block the
                # DVE pooling stream (ACT is idle in this window)
                nc.scalar.activation(out=xfb_rem[cg][:], in_=rem_raw[:],
                                     func=mybir.ActivationFunctionType.Copy,
                                     scale=POOL_SCALE)
                nc.tensor.matmul(kr_ps[:], wkT[cg][:], xfb_rem[cg][:],
                                 start=(cg == 0), stop=False)

            for cg in range(CG):
                for ib in range(IL):
                    x1 = p1.tile([128, DS, W], f32, tag="x1")
                    nc.sync.dma_start(
                        out=x1[:],
                        in_=x_ext.ap()[cg * 128:(cg + 1) * 128,
                                       ib * DS:(ib + 1) * DS, :])
                    nc.vector.tensor_reduce(
                        out=xf[cg][:, ib * WP:(ib + 1) * WP],
                        in_=x1[:].rearrange("p h (j z) -> p j h z", z=DS),
                        axis=AXY, op=ADD)
                    # stage completed quarters into the bounce buffer so the
                    # collective fires the moment the last slice lands
                    if ib % 4 == 3:
                        nc.gpsimd.dma_start(
                            out=xf_loc_d[cg][:, (ib - 3) * WP:(ib + 1) * WP],
                            in_=xf[cg][:, (ib - 3) * WP:(ib + 1) * WP])

                # local bf16 copy (applies the 1/64 pooling scale); on DVE so it
                # slots right behind this group's own pooling ADDs
                nc.vector.tensor_scalar_mul(xfb_loc[cg][:], xf[cg][:], POOL_SCALE)
                # local q/k partials (overlap the exchange)
                nc.tensor.matmul(q_ps[:], wqT[cg][:], xfb_loc[cg][:],
                                 start=(cg == 0), stop=False)
                nc.tensor.matmul(kl_ps[:], wkT[cg][:], xfb_loc[cg][:],
                                 start=(cg == 0), stop=False)
                nc.gpsimd.collective_compute(
                    "AllGather",
                    mybir.AluOpType.bypass,
                    ins=[xf_loc_d[cg].opt()],
                    outs=[xf_all_d[cg].opt()],
                    replica_groups=[[0, 1], [2, 3], [4, 5], [6, 7]],
                )
                if cg > 0:
                    remote_recover(cg - 1)

            remote_recover(CG - 1)

            # ================= LOCAL attention half =================
            # Everything below up to the "REMOTE" marker depends only on local
            # pooled data, so it executes while the last AllGather is in flight.
            nc.tensor.matmul(q_ps[:], bq_b[:], ones[:, :NLOC], start=False, stop=True)
            q_sb = persist.tile([K, NLOC], bf16, tag="q_sb")
            nc.vector.tensor_copy(out=q_sb[:], in_=q_ps[:])
            nc.tensor.matmul(kl_ps[:], bk_b[:], ones[:, :NLOC], start=False, stop=True)
            k_loc = persist.tile([K, NLOC], bf16, tag="k_loc")
            nc.vector.tensor_copy(out=k_loc[:], in_=kl_ps[:])

            vT = [persist.tile([128, C], bf16, tag=f"vT{nt}", name=f"vT{nt}")
                  for nt in range(NT)]

            def vt_tile(nt):
                src = xfb_loc if nt < 4 else xfb_rem
                j = nt % 4
                v_ps = psA.tile([128, C], f32, tag="s", name=f"v_ps{nt}")
                for cg in range(CG):
                    nc.tensor.matmul(v_ps[:], src[cg][:, j * 128:(j + 1) * 128],
                                     wvT[cg][:], start=(cg == 0), stop=False)
                nc.tensor.matmul(v_ps[:], ones[:, :128], bv_b[:], start=False, stop=True)
                nc.vector.tensor_copy(out=vT[nt][:], in_=v_ps[:])

            for nt in range(4):
                vt_tile(nt)

            # attn holds UNNORMALIZED exp(e/sqrt(K)); normalization is applied to
            # y at the very end via a row-sum rescale. Energies are tiny for this
            # model (|e/sqrt(K)| < ~0.05), so exp without max-subtraction is safe.
            attn = [persist.tile([128, N], bf16, tag=f"attn{mt}", name=f"attn{mt}")
                    for mt in range(4)]
            k_rem = persist.tile([K, NLOC], bf16, tag="k_rem")

            def energy_half(mt, half):
                ksb = k_loc if half == 0 else k_rem
                e_ps = psA.tile([128, NLOC], f32, tag="s", name=f"e_ps{mt}_{half}")
                nc.tensor.matmul(e_ps[:], q_sb[:, mt * 128:(mt + 1) * 128], ksb[:],
                                 start=True, stop=True)
                nc.scalar.activation(out=attn[mt][:, half * NLOC:(half + 1) * NLOC],
                                     in_=e_ps[:], func=Exp, scale=K ** -0.5)

            for mt in range(4):
                energy_half(mt, 0)

            attnT = [persist.tile([128, NLOC], bf16, tag=f"attnT{nt}", name=f"attnT{nt}")
                     for nt in range(NT)]

            def attn_t(nt):
                at_ps = psA.tile([128, NLOC], bf16, tag="s", name=f"at_ps{nt}")
                for mt in range(4):
                    nc.tensor.transpose(at_ps[:, mt * 128:(mt + 1) * 128],
                                        attn[mt][:, nt * 128:(nt + 1) * 128],
                                        ident[:])
                nc.vector.tensor_copy(out=attnT[nt][:], in_=at_ps[:])

            for nt in range(4):
                attn_t(nt)

            # y_raw[d, m] = sum_n v[d, n] exp[m, n]; rowsum[m] = sum_n exp[m, n]
            y_ps = [psY.tile([128, NLOC], f32, tag=f"y{dt}", name=f"yps{dt}")
                    for dt in range(CG)]
            rs_ps = psA.tile([1, NLOC], f32, tag="s", name="rs_ps")
            for nt in range(4):
                for dt in range(CG):
                    nc.tensor.matmul(y_ps[dt][:], vT[nt][:, dt * 128:(dt + 1) * 128],
                                     attnT[nt][:], start=(nt == 0), stop=False)
                nc.tensor.matmul(rs_ps[:], ones_col[:], attnT[nt][:],
                                 start=(nt == 0), stop=False)

            # ================= REMOTE attention half =================
            nc.tensor.matmul(kr_ps[:], bk_b[:], ones[:, :NLOC], start=False, stop=True)
            nc.vector.tensor_copy(out=k_rem[:], in_=kr_ps[:])

            for nt in range(4, NT):
                vt_tile(nt)
            for mt in range(4):
                energy_half(mt, 1)
            for nt in range(4, NT):
                attn_t(nt)
            for nt in range(4, NT):
                for dt in range(CG):
                    nc.tensor.matmul(y_ps[dt][:], vT[nt][:, dt * 128:(dt + 1) * 128],
                                     attnT[nt][:], start=False, stop=(nt == NT - 1))
                nc.tensor.matmul(rs_ps[:], ones_col[:], attnT[nt][:],
                                 start=False, stop=(nt == NT - 1))

            # softmax denominators -> broadcast rescale of y
            rinv_row = persist.tile([1, NLOC], f32, tag="rinv_row")
            nc.vector.reciprocal(rinv_row[:], rs_ps[:])
            rb_ps = psA.tile([128, NLOC], f32, tag="s")
            nc.tensor.matmul(rb_ps[:], ones_f32[:], rinv_row[:], start=True, stop=True)
            rb_sb = persist.tile([128, NLOC], f32, tag="rb_sb")
            nc.vector.tensor_copy(out=rb_sb[:], in_=rb_ps[:])

            y = [persist.tile([128, NLOC], f32, tag=f"y{dt}", name=f"y{dt}")
                 for dt in range(CG)]
            for dt in range(CG):
                nc.vector.tensor_tensor(out=y[dt][:], in0=y_ps[dt][:], in1=rb_sb[:],
                                        op=MUL)

            # ---- phase 3: out = x + upsample8(y) ----
            # loads on sync ring, adds on DVE, stores on scalar ring
            for cg in range(CG):
                for ib in range(NCHUNK):
                    x3 = p3.tile([128, ROWS_PER_CHUNK, W], f32, tag="x3")
                    nc.sync.dma_start(
                        out=x3[:],
                        in_=x_ext.ap()[cg * 128:(cg + 1) * 128,
                                       ib * ROWS_PER_CHUNK:(ib + 1) * ROWS_PER_CHUNK, :])
                    for i in range(2):
                        xv = x3[:, i * DS:(i + 1) * DS, :] \
                            .rearrange("p h (j z) -> p h j z", z=DS)
                        yv = y[cg][:, (ib * 2 + i) * WP:(ib * 2 + i + 1) * WP] \
                            [:, None, :, None].broadcast_to([128, DS, WP, DS])
                        nc.vector.tensor_tensor(out=xv, in0=xv, in1=yv, op=ADD)
                    nc.scalar.dma_start(
                        out=out_ext.ap()[cg * 128:(cg + 1) * 128,
                                         ib * ROWS_PER_CHUNK:(ib + 1) * ROWS_PER_CHUNK, :],
                        in_=x3[:])

    nc.finalize()
    return nc


def _get_nc():
    if "nc" not in _CACHE:
        _CACHE["nc"] = _build()
    return _CACHE["nc"]


def kernel(x, Wq, bq, Wk, bk, Wv, bv):
    global LAST_EXEC_NS
    from concourse.bass_utils import run_bass_kernel_spmd

    x = np.asarray(x, dtype=np.float32)
    Wq = np.asarray(Wq, dtype=np.float32)
    bq = np.asarray(bq, dtype=np.float32).reshape(1, K)
    Wk = np.asarray(Wk, dtype=np.float32)
    bk = np.asarray(bk, dtype=np.float32).reshape(1, K)
    Wv = np.asarray(Wv, dtype=np.float32)
    bv = np.asarray(bv, dtype=np.float32).reshape(1, C)

    nc = _get_nc()
    in_maps = []
    for core in range(8):
        b, half = core // 2, core % 2
        in_maps.append({
            "x": np.ascontiguousarray(x[b, :, half * HL:(half + 1) * HL, :]),
            "wq": Wq, "bq": bq, "wk": Wk, "bk": bk, "wv": Wv, "bv": bv,
        })

    res = run_bass_kernel_spmd(nc, in_maps, core_ids=list(range(8)), trace=TRACE)
    LAST_EXEC_NS = res.exec_time_ns

    out = np.empty((B, C, H, W), dtype=np.float32)
    for core in range(8):
        b, half = core // 2, core % 2
        out[b, :, half * HL:(half + 1) * HL, :] = res.results[core]["out"]
    return out

